# revision 10
# baseline (speedup 1.0000x reference)
"""GCN encoder (nn_GCNEncoder) Trainium2 Bass kernel.

Math: with a fully-connected graph + self loops, gcn_norm gives the uniform
adjacency A = 1/N. Then A @ X broadcasts mean_n(X) to every node, so after
layer 1 the node features are constant within each graph and the whole GCN
collapses to a per-graph vector chain:

  locbar[b] = mean_n locs[b, n, :]                       (R^2)
  g0[b]     = locbar[b] @ W_init + b_init                (R^D)
  g1        = relu(g0 @ Ws[0] + bs[0])
  g2        = relu(g1 @ Ws[1] + bs[1])
  g3        = g2 @ Ws[2] + bs[2]
  init_h[b, n, :]  = locs[b, n, :] @ W_init + b_init
  h_final[b, n, :] = init_h[b, n, :] + g3[b, :]

Outputs (h_final, init_h) are 2 x [2048, 100, 128] = 105 M elements -> the
kernel is store-bandwidth bound. Both outputs are stored as bf16 (upcast to
f32 on the host): output rounding is <= 2^-9 relative (~2e-3 under the
absmax metric, gate is 2e-2), and store traffic halves to 13.1 MB/core
(~37 us at 358 GB/s/core).

Device layout (per core: 256 graphs = 25600 tokens), all FEATURE-major:
 - Token column index c = u*128 + p with u in [0,200), p in [0,128):
   graph = p + 128*(u >= 100), node = u mod 100. Host packs `master2`
   [10, 25600] bf16 accordingly (locs hi/lo x/y rows + ones rows).
 - PE: out[d, c] tiles [128, 512] = matmul(lhsT=rhsW [10,128] stationary,
   rhs=master2[:, 512j:512j+512] moving) -> init_h in PSUM. K=10 rows carry
   the f32->bf16 hi/lo decomposition of locs and W_init (exact products,
   only lo*lo cross terms dropped, ~2^-18).
 - Within one tile every column c has graph = (c%128) + 128h (h = j>=25),
   so h_final = psum + g3rep[h] is ONE VectorE tensor_tensor add with a
   tile-constant f32 operand ([128, 512] = per-half g3 repeated 4x).
   ScalarE evacuates init_h (AF.Copy, bf16 out). No sel-matmul, no
   transposes: the g3 chain is computed feature-major natively.
 - Stores: [128, 2560] bf16 strips (5120 B/partition contiguous), outF on
   the sync ring, outI on the scalar ring (~6.55 MB each way).
Host unpacks (d, c) -> (b, n, d) and upcasts to f32.
"""

import numpy as np
from contextlib import ExitStack

import concourse.bass as bass
import concourse.mybir as mybir
import concourse.tile as tile
from concourse.bass_utils import run_bass_kernel_spmd

F32 = mybir.dt.float32
BF16 = mybir.dt.bfloat16
AF = mybir.ActivationFunctionType

B, N, D, L = 2048, 100, 128, 3
NCORES = 8
BG = B // NCORES          # 256 graphs per core
T = BG * N                # 25600 tokens per core
NU = T // 128             # 200 token columns of 128 (u index)
NJ = NU // 4              # 50 psum tiles of [128, 512]
JPS = 5                   # psum tiles per store strip
NS = NJ // JPS            # store strips of [128, 512*JPS]
SBUFS = 10                # strip buffers per output
KB = 10                   # contraction rows (locs hi/lo x/y + ones)


def _split_multiwaits(nc, max_waits=1):
    """The walrus build in this container rejects instructions carrying more
    than one sync-wait command. Split extras into single-wait NoOps inserted
    immediately before the instruction (same engine, so sequencer order
    preserves semantics exactly)."""
    cnt = 0
    for f in nc.m.functions:
        for b in f.blocks:
            il = b.instructions
            i = 0
            while i < len(il):
                ins = il[i]
                si = ins.sync_info
                if si is not None and si.on_wait and len(si.on_wait) > max_waits:
                    waits = list(si.on_wait)
                    for w in waits[:-max_waits]:
                        nop = mybir.InstNoOp(name=f"I-SWAIT-{cnt}", ins=[], outs=[])
                        cnt += 1
                        nop.engine = ins.engine
                        nop.sync_info = mybir.SyncInfo(on_wait=[w], on_update=[])
                        il.insert(i, nop)
                        i += 1
                    ins.sync_info = mybir.SyncInfo(
                        on_wait=waits[-max_waits:],
                        on_update=list(si.on_update or []))
                i += 1
    return cnt


def _build_program(split=True, reps=1):
    nc = bass.Bass("TRN2", target_bir_lowering=False, debug=False,
                   num_devices=NCORES)

    ins = {}
    for name, shape, dt in [
        ("master2", [KB, T], BF16),
        ("rhsW", [KB, D], BF16),
        ("locs_gm", [BG, 2 * N], F32),
        ("wmean", [2, D], F32),
        ("bcol", [D, 1], F32),
        ("bsT", [D, L], F32),
        ("Ws", [L, D, D], F32),
        ("ident", [D, D], F32),
    ]:
        ins[name] = nc.dram_tensor(name, shape, dt, kind="ExternalInput").ap()

    out_final = nc.dram_tensor("out_final", [D, T], BF16, kind="ExternalOutput").ap()
    out_init = nc.dram_tensor("out_init", [D, T], BF16, kind="ExternalOutput").ap()
    outF_r = out_final.rearrange("d (s c) -> s d c", s=NS)
    outI_r = out_init.rearrange("d (s c) -> s d c", s=NS)

    with tile.TileContext(nc) as tc, ExitStack() as ctx:
        const = ctx.enter_context(tc.tile_pool(name="const", bufs=1))

        ident_sb = const.tile([D, D], F32, tag="ident")
        nc.sync.dma_start(ident_sb[:], ins["ident"][:])
        wmean_sb = const.tile([2, D], F32, tag="wmean")
        nc.sync.dma_start(wmean_sb[:], ins["wmean"][:])
        bcol_sb = const.tile([D, 1], F32, tag="bcol")
        nc.sync.dma_start(bcol_sb[:], ins["bcol"][:])
        bsT_sb = const.tile([D, L], F32, tag="bsT")
        nc.sync.dma_start(bsT_sb[:], ins["bsT"][:])
        ws_sb = []
        for l in range(L):
            w = const.tile([D, D], F32, tag=f"ws{l}")
            nc.sync.dma_start(w[:], ins["Ws"][l])
            ws_sb.append(w)
        rhsW_sb = const.tile([KB, D], BF16, tag="rhsW")
        nc.sync.dma_start(rhsW_sb[:], ins["rhsW"][:])
        master_sb = const.tile([KB, T], BF16, tag="master")
        nc.scalar.dma_start(master_sb[:], ins["master2"][:])

        # per-half g3 (feature-major), repeated 4x along free axis so the
        # broadcast add is a plain [128, 512] operand
        g3rep = [const.tile([D, 512], F32, tag=f"g3rep{h}", name=f"g3rep{h}")
                 for h in range(2)]

        # ---------------- per-graph g3 chain (fp32, feature-major) --------
        with tc.tile_pool(name="gps", bufs=2, space="PSUM") as gps, \
             tc.tile_pool(name="gtmp", bufs=2) as gtmp:
            for h in range(2):
                hs = slice(128 * h, 128 * (h + 1))
                lg = gtmp.tile([128, 2 * N], F32, tag="lg")
                nc.sync.dma_start(lg[:], ins["locs_gm"][hs, :])
                lb = gtmp.tile([128, 2], F32, tag="lb")
                lgk = lg[:].rearrange("p (n k) -> p k n", k=2)
                for k in range(2):
                    nc.vector.tensor_reduce(
                        lb[:, k:k + 1], lgk[:, k:k + 1, :],
                        axis=mybir.AxisListType.X, op=mybir.AluOpType.add)
                tp = gps.tile([2, 128], F32, tag="tp")
                nc.tensor.transpose(tp[:], lb[:], ident_sb[:])
                lbT = gtmp.tile([2, 128], F32, tag="lbT")
                nc.vector.tensor_copy(lbT[:], tp[:])

                mp = gps.tile([128, 128], F32, tag="mp")
                nc.tensor.matmul(mp[:], wmean_sb[:], lbT[:],
                                 start=True, stop=True)
                g_prev = gtmp.tile([128, 128], F32, tag=f"g0h{h}")
                nc.scalar.activation(g_prev[:], mp[:], AF.Identity,
                                     bias=bcol_sb[:, 0:1])
                for l in range(L):
                    pp = gps.tile([128, 128], F32, tag="mp")
                    nc.tensor.matmul(pp[:], ws_sb[l][:], g_prev[:],
                                     start=True, stop=True)
                    g_next = gtmp.tile([128, 128], F32, tag=f"g{l + 1}h{h}")
                    nc.scalar.activation(
                        g_next[:], pp[:], AF.Relu if l < L - 1 else AF.Identity,
                        bias=bsT_sb[:, l:l + 1])
                    g_prev = g_next
                for r in range(4):
                    nc.vector.tensor_copy(g3rep[h][:, 128 * r:128 * (r + 1)],
                                          g_prev[:])

        # ---------------- main loop ----------------
        pspool = ctx.enter_context(tc.tile_pool(name="ps", bufs=8, space="PSUM"))
        sFpool = ctx.enter_context(tc.tile_pool(name="sF", bufs=SBUFS))
        sIpool = ctx.enter_context(tc.tile_pool(name="sI", bufs=SBUFS))

        def main_loop():
            main_body(nc, tc, master_sb, rhsW_sb, g3rep, pspool, sFpool,
                      sIpool, outF_r, outI_r)

        if reps > 1:
            with tc.For_i(0, reps, 1):
                main_loop()
        else:
            main_loop()

    if split:
        _split_multiwaits(nc)
    return nc


def main_body(nc, tc, master_sb, rhsW_sb, g3rep, pspool, sFpool, sIpool,
              outF_r, outI_r):
    sF = sI = None
    for j in range(NJ):
        ps = pspool.tile([128, 512], F32, tag="ps")
        nc.tensor.matmul(ps[:], rhsW_sb[:], master_sb[:, 512 * j:512 * (j + 1)],
                         start=True, stop=True)
        s, q, h = j // JPS, j % JPS, j // (NJ // 2)
        if q == 0:
            sF = sFpool.tile([128, 512 * JPS], BF16, tag="sF")
            sI = sIpool.tile([128, 512 * JPS], BF16, tag="sI")
        nc.vector.tensor_tensor(sF[:, 512 * q:512 * (q + 1)], ps[:],
                                g3rep[h][:], op=mybir.AluOpType.add)
        nc.scalar.activation(sI[:, 512 * q:512 * (q + 1)], ps[:], AF.Copy)
        if q == JPS - 1:
            nc.sync.dma_start(outF_r[s], sF[:])
            nc.scalar.dma_start(outI_r[s], sI[:])


def _bf_split(x, n=2):
    import ml_dtypes
    outs = []
    r = np.asarray(x, dtype=np.float32)
    for _ in range(n):
        h = r.astype(ml_dtypes.bfloat16)
        outs.append(h)
        r = r - h.astype(np.float32)
    return outs


def _prep_core_inputs(locs, W_init, b_init, Ws, bs):
    """Host-side shard + constant prep. Returns list of per-core input maps."""
    import ml_dtypes
    bfdt = ml_dtypes.bfloat16
    locs = np.ascontiguousarray(locs, dtype=np.float32)
    W_init = np.asarray(W_init, dtype=np.float32)
    b_init = np.asarray(b_init, dtype=np.float32)
    Ws = np.ascontiguousarray(Ws, dtype=np.float32)
    bs = np.asarray(bs, dtype=np.float32)

    Wh, Wl = _bf_split(W_init)
    bh, bl = _bf_split(b_init)
    rhs_rows = [Wh[0], Wh[1], Wl[0], Wl[1], Wh[0], Wh[1], Wl[0], Wl[1], bh, bl]
    rhsW = np.ascontiguousarray(np.stack(rhs_rows).astype(bfdt))

    wmean = np.ascontiguousarray(W_init / np.float32(N))
    bcol = np.ascontiguousarray(b_init.reshape(D, 1))
    bsT = np.ascontiguousarray(bs.T)
    ident = np.eye(D, dtype=np.float32)

    in_maps = []
    for k in range(NCORES):
        lc = locs[BG * k:BG * (k + 1)]          # [256, 100, 2]
        # token column c = (h*100 + n)*128 + p  ->  graph h*128+p, node n
        xs = lc.reshape(2, 128, N, 2).transpose(0, 2, 1, 3).reshape(T, 2)
        lx, ly = xs[:, 0], xs[:, 1]
        lxh, lxl = _bf_split(lx)
        lyh, lyl = _bf_split(ly)
        ones = np.ones(T, dtype=bfdt)
        master = np.stack([lxh, lyh, lxh, lyh, lxl, lyl, lxl, lyl, ones, ones])
        in_maps.append({
            "master2": np.ascontiguousarray(master.astype(bfdt)),
            "rhsW": rhsW,
            "locs_gm": np.ascontiguousarray(lc.reshape(BG, 2 * N)),
            "wmean": wmean,
            "bcol": bcol,
            "bsT": bsT,
            "Ws": Ws,
            "ident": ident,
        })
    return in_maps


def _unpack_core(arr):
    """[D, T] (d, c) bf16 -> [BG, N, D] f32, c = (h*100+n)*128+p, b = h*128+p."""
    a = np.asarray(arr).astype(np.float32)
    return a.reshape(D, 2, N, 128).transpose(1, 3, 2, 0).reshape(BG, N, D)


_CACHED_NC = None


def _get_nc():
    global _CACHED_NC
    if _CACHED_NC is None:
        _CACHED_NC = _build_program()
    return _CACHED_NC


def kernel(locs, W_init, b_init, Ws, bs, _trace=False):
    nc = _get_nc()
    in_maps = _prep_core_inputs(locs, W_init, b_init, Ws, bs)
    res = run_bass_kernel_spmd(nc, in_maps, list(range(NCORES)), trace=_trace)
    h = np.concatenate(
        [_unpack_core(res.results[k]["out_final"]) for k in range(NCORES)],
        axis=0)
    init_h = np.concatenate(
        [_unpack_core(res.results[k]["out_init"]) for k in range(NCORES)],
        axis=0)
    if _trace:
        return (h, init_h), res
    return (h, init_h)


# revision 11
# speedup vs baseline: 1.0291x; 1.0291x over previous
"""GCN encoder (nn_GCNEncoder) Trainium2 Bass kernel.

Math: with a fully-connected graph + self loops, gcn_norm gives the uniform
adjacency A = 1/N. Then A @ X broadcasts mean_n(X) to every node, so after
layer 1 the node features are constant within each graph and the whole GCN
collapses to a per-graph vector chain:

  locbar[b] = mean_n locs[b, n, :]                       (R^2)
  g0[b]     = locbar[b] @ W_init + b_init                (R^D)
  g1        = relu(g0 @ Ws[0] + bs[0])
  g2        = relu(g1 @ Ws[1] + bs[1])
  g3        = g2 @ Ws[2] + bs[2]
  init_h[b, n, :]  = locs[b, n, :] @ W_init + b_init
  h_final[b, n, :] = init_h[b, n, :] + g3[b, :]

Outputs (h_final, init_h) are 2 x [2048, 100, 128] = 105 M elements -> the
kernel is store-bandwidth bound. Both outputs are stored as bf16 (upcast to
f32 on the host): output rounding is <= 2^-9 relative (~2e-3 under the
absmax metric, gate is 2e-2), and store traffic halves to 13.1 MB/core
(~37 us at 358 GB/s/core).

Device layout (per core: 256 graphs = 25600 tokens), all FEATURE-major:
 - Token column index c = u*128 + p with u in [0,200), p in [0,128):
   graph = p + 128*(u >= 100), node = u mod 100. Host packs `master2`
   [10, 25600] bf16 accordingly (locs hi/lo x/y rows + ones rows).
 - PE: out[d, c] tiles [128, 512] = matmul(lhsT=rhsW [10,128] stationary,
   rhs=master2[:, 512j:512j+512] moving) -> init_h in PSUM. K=10 rows carry
   the f32->bf16 hi/lo decomposition of locs and W_init (exact products,
   only lo*lo cross terms dropped, ~2^-18).
 - Within one tile every column c has graph = (c%128) + 128h (h = j>=25),
   so h_final = psum + g3rep[h] is ONE VectorE tensor_tensor add with a
   tile-constant f32 operand ([128, 512] = per-half g3 repeated 4x).
   ScalarE evacuates init_h (AF.Copy, bf16 out). No sel-matmul, no
   transposes: the g3 chain is computed feature-major natively.
 - Stores: [128, 2560] bf16 strips (5120 B/partition contiguous), outF on
   the sync ring, outI on the scalar ring (~6.55 MB each way).
Host unpacks (d, c) -> (b, n, d) and upcasts to f32.
"""

import numpy as np
from contextlib import ExitStack

import concourse.bass as bass
import concourse.mybir as mybir
import concourse.tile as tile
from concourse.bass_utils import run_bass_kernel_spmd

F32 = mybir.dt.float32
BF16 = mybir.dt.bfloat16
AF = mybir.ActivationFunctionType

B, N, D, L = 2048, 100, 128, 3
NCORES = 8
BG = B // NCORES          # 256 graphs per core
T = BG * N                # 25600 tokens per core
NU = T // 128             # 200 token columns of 128 (u index)
NJ = NU // 4              # 50 psum tiles of [128, 512]
JPS = 5                   # psum tiles per store strip
NS = NJ // JPS            # store strips of [128, 512*JPS]
SBUFS = 8                 # strip buffers per output
KB = 10                   # contraction rows (locs hi/lo x/y + ones)


def _split_multiwaits(nc, max_waits=1):
    """The walrus build in this container rejects instructions carrying more
    than one sync-wait command. Split extras into single-wait NoOps inserted
    immediately before the instruction (same engine, so sequencer order
    preserves semantics exactly)."""
    cnt = 0
    for f in nc.m.functions:
        for b in f.blocks:
            il = b.instructions
            i = 0
            while i < len(il):
                ins = il[i]
                si = ins.sync_info
                if si is not None and si.on_wait and len(si.on_wait) > max_waits:
                    waits = list(si.on_wait)
                    for w in waits[:-max_waits]:
                        nop = mybir.InstNoOp(name=f"I-SWAIT-{cnt}", ins=[], outs=[])
                        cnt += 1
                        nop.engine = ins.engine
                        nop.sync_info = mybir.SyncInfo(on_wait=[w], on_update=[])
                        il.insert(i, nop)
                        i += 1
                    ins.sync_info = mybir.SyncInfo(
                        on_wait=waits[-max_waits:],
                        on_update=list(si.on_update or []))
                i += 1
    return cnt


def _build_program(split=True, reps=1):
    nc = bass.Bass("TRN2", target_bir_lowering=False, debug=False,
                   num_devices=NCORES)

    ins = {}
    for name, shape, dt in [
        ("master2", [KB, T], BF16),
        ("rhsW", [KB, D], BF16),
        ("locs_gm", [BG, 2 * N], F32),
        ("wmean", [2, D], F32),
        ("bcol", [D, 1], F32),
        ("bsT", [D, L], F32),
        ("Ws", [L, D, D], F32),
        ("ident", [D, D], F32),
    ]:
        ins[name] = nc.dram_tensor(name, shape, dt, kind="ExternalInput").ap()

    out_final = nc.dram_tensor("out_final", [D, T], BF16, kind="ExternalOutput").ap()
    out_init = nc.dram_tensor("out_init", [D, T], BF16, kind="ExternalOutput").ap()
    outF_r = out_final.rearrange("d (s c) -> s d c", s=NS)
    outI_r = out_init.rearrange("d (s c) -> s d c", s=NS)

    with tile.TileContext(nc) as tc, ExitStack() as ctx:
        const = ctx.enter_context(tc.tile_pool(name="const", bufs=1))

        ident_sb = const.tile([D, D], F32, tag="ident")
        nc.sync.dma_start(ident_sb[:], ins["ident"][:])
        wmean_sb = const.tile([2, D], F32, tag="wmean")
        nc.sync.dma_start(wmean_sb[:], ins["wmean"][:])
        bcol_sb = const.tile([D, 1], F32, tag="bcol")
        nc.sync.dma_start(bcol_sb[:], ins["bcol"][:])
        bsT_sb = const.tile([D, L], F32, tag="bsT")
        nc.sync.dma_start(bsT_sb[:], ins["bsT"][:])
        ws_sb = []
        for l in range(L):
            w = const.tile([D, D], F32, tag=f"ws{l}")
            nc.sync.dma_start(w[:], ins["Ws"][l])
            ws_sb.append(w)
        rhsW_sb = const.tile([KB, D], BF16, tag="rhsW")
        nc.sync.dma_start(rhsW_sb[:], ins["rhsW"][:])
        master_sb = const.tile([KB, T], BF16, tag="master")
        mchunk = T // NS
        for s in range(NS):
            nc.scalar.dma_start(master_sb[:, mchunk * s:mchunk * (s + 1)],
                                ins["master2"][:, mchunk * s:mchunk * (s + 1)])

        # per-half g3 (feature-major), repeated 4x along free axis so the
        # broadcast add is a plain [128, 512] operand
        g3rep = [const.tile([D, 512], F32, tag=f"g3rep{h}", name=f"g3rep{h}")
                 for h in range(2)]

        # ---------------- per-graph g3 chain (fp32, feature-major) --------
        with tc.tile_pool(name="gps", bufs=2, space="PSUM") as gps, \
             tc.tile_pool(name="gtmp", bufs=2) as gtmp:
            for h in range(2):
                hs = slice(128 * h, 128 * (h + 1))
                lg = gtmp.tile([128, 2 * N], F32, tag="lg")
                nc.sync.dma_start(lg[:], ins["locs_gm"][hs, :])
                lb = gtmp.tile([128, 2], F32, tag="lb")
                lgk = lg[:].rearrange("p (n k) -> p k n", k=2)
                for k in range(2):
                    nc.vector.tensor_reduce(
                        lb[:, k:k + 1], lgk[:, k:k + 1, :],
                        axis=mybir.AxisListType.X, op=mybir.AluOpType.add)
                tp = gps.tile([2, 128], F32, tag="tp")
                nc.tensor.transpose(tp[:], lb[:], ident_sb[:])
                lbT = gtmp.tile([2, 128], F32, tag="lbT")
                nc.vector.tensor_copy(lbT[:], tp[:])

                mp = gps.tile([128, 128], F32, tag="mp")
                nc.tensor.matmul(mp[:], wmean_sb[:], lbT[:],
                                 start=True, stop=True)
                g_prev = gtmp.tile([128, 128], F32, tag=f"g0h{h}")
                nc.scalar.activation(g_prev[:], mp[:], AF.Identity,
                                     bias=bcol_sb[:, 0:1])
                for l in range(L):
                    pp = gps.tile([128, 128], F32, tag="mp")
                    nc.tensor.matmul(pp[:], ws_sb[l][:], g_prev[:],
                                     start=True, stop=True)
                    g_next = gtmp.tile([128, 128], F32, tag=f"g{l + 1}h{h}")
                    nc.scalar.activation(
                        g_next[:], pp[:], AF.Relu if l < L - 1 else AF.Identity,
                        bias=bsT_sb[:, l:l + 1])
                    g_prev = g_next
                for r in range(4):
                    nc.vector.tensor_copy(g3rep[h][:, 128 * r:128 * (r + 1)],
                                          g_prev[:])

        # ---------------- main loop ----------------
        pspool = ctx.enter_context(tc.tile_pool(name="ps", bufs=8, space="PSUM"))
        sFpool = ctx.enter_context(tc.tile_pool(name="sF", bufs=SBUFS))
        sIpool = ctx.enter_context(tc.tile_pool(name="sI", bufs=SBUFS))

        def main_loop():
            main_body(nc, tc, master_sb, rhsW_sb, g3rep, pspool, sFpool,
                      sIpool, outF_r, outI_r)

        if reps > 1:
            with tc.For_i(0, reps, 1):
                main_loop()
        else:
            main_loop()

    if split:
        _split_multiwaits(nc)
    return nc


def main_body(nc, tc, master_sb, rhsW_sb, g3rep, pspool, sFpool, sIpool,
              outF_r, outI_r):
    sF = sI = None
    for j in range(NJ):
        ps = pspool.tile([128, 512], F32, tag="ps")
        nc.tensor.matmul(ps[:], rhsW_sb[:], master_sb[:, 512 * j:512 * (j + 1)],
                         start=True, stop=True)
        s, q, h = j // JPS, j % JPS, j // (NJ // 2)
        if q == 0:
            sF = sFpool.tile([128, 512 * JPS], BF16, tag="sF")
            sI = sIpool.tile([128, 512 * JPS], BF16, tag="sI")
        nc.vector.tensor_tensor(sF[:, 512 * q:512 * (q + 1)], ps[:],
                                g3rep[h][:], op=mybir.AluOpType.add)
        nc.scalar.activation(sI[:, 512 * q:512 * (q + 1)], ps[:], AF.Copy)
        if q == JPS - 1:
            nc.sync.dma_start(outF_r[s], sF[:])
            nc.scalar.dma_start(outI_r[s], sI[:])


def _bf_split(x, n=2):
    import ml_dtypes
    outs = []
    r = np.asarray(x, dtype=np.float32)
    for _ in range(n):
        h = r.astype(ml_dtypes.bfloat16)
        outs.append(h)
        r = r - h.astype(np.float32)
    return outs


def _prep_core_inputs(locs, W_init, b_init, Ws, bs):
    """Host-side shard + constant prep. Returns list of per-core input maps."""
    import ml_dtypes
    bfdt = ml_dtypes.bfloat16
    locs = np.ascontiguousarray(locs, dtype=np.float32)
    W_init = np.asarray(W_init, dtype=np.float32)
    b_init = np.asarray(b_init, dtype=np.float32)
    Ws = np.ascontiguousarray(Ws, dtype=np.float32)
    bs = np.asarray(bs, dtype=np.float32)

    Wh, Wl = _bf_split(W_init)
    bh, bl = _bf_split(b_init)
    rhs_rows = [Wh[0], Wh[1], Wl[0], Wl[1], Wh[0], Wh[1], Wl[0], Wl[1], bh, bl]
    rhsW = np.ascontiguousarray(np.stack(rhs_rows).astype(bfdt))

    wmean = np.ascontiguousarray(W_init / np.float32(N))
    bcol = np.ascontiguousarray(b_init.reshape(D, 1))
    bsT = np.ascontiguousarray(bs.T)
    ident = np.eye(D, dtype=np.float32)

    in_maps = []
    for k in range(NCORES):
        lc = locs[BG * k:BG * (k + 1)]          # [256, 100, 2]
        # token column c = (h*100 + n)*128 + p  ->  graph h*128+p, node n
        xs = lc.reshape(2, 128, N, 2).transpose(0, 2, 1, 3).reshape(T, 2)
        lx, ly = xs[:, 0], xs[:, 1]
        lxh, lxl = _bf_split(lx)
        lyh, lyl = _bf_split(ly)
        ones = np.ones(T, dtype=bfdt)
        master = np.stack([lxh, lyh, lxh, lyh, lxl, lyl, lxl, lyl, ones, ones])
        in_maps.append({
            "master2": np.ascontiguousarray(master.astype(bfdt)),
            "rhsW": rhsW,
            "locs_gm": np.ascontiguousarray(lc.reshape(BG, 2 * N)),
            "wmean": wmean,
            "bcol": bcol,
            "bsT": bsT,
            "Ws": Ws,
            "ident": ident,
        })
    return in_maps


def _unpack_core(arr):
    """[D, T] (d, c) bf16 -> [BG, N, D] f32, c = (h*100+n)*128+p, b = h*128+p."""
    a = np.asarray(arr).astype(np.float32)
    return a.reshape(D, 2, N, 128).transpose(1, 3, 2, 0).reshape(BG, N, D)


_CACHED_NC = None


def _get_nc():
    global _CACHED_NC
    if _CACHED_NC is None:
        _CACHED_NC = _build_program()
    return _CACHED_NC


def kernel(locs, W_init, b_init, Ws, bs, _trace=False):
    nc = _get_nc()
    in_maps = _prep_core_inputs(locs, W_init, b_init, Ws, bs)
    res = run_bass_kernel_spmd(nc, in_maps, list(range(NCORES)), trace=_trace)
    h = np.concatenate(
        [_unpack_core(res.results[k]["out_final"]) for k in range(NCORES)],
        axis=0)
    init_h = np.concatenate(
        [_unpack_core(res.results[k]["out_init"]) for k in range(NCORES)],
        axis=0)
    if _trace:
        return (h, init_h), res
    return (h, init_h)
